# revision 23
# baseline (speedup 1.0000x reference)
"""Neighbourhood attention block (7x7 clamped window) on 8 Trainium2 cores.

Sharding: (batch, head-pair) tensor parallel. Core c handles batch b = c//4
and heads (2*(c%4), 2*(c%4)+1). Each core computes q/k/v projections for its
two heads, neighbourhood attention, and a partial output projection; host
sums the 4 bf16 partials per batch in fp32.

v2 layout: all matmul operands bf16 (FWL weight loads, halved DMA).
Scores stay in scoresT [key, query] tiles, two tiles paired per 2-bank PSUM
so one Exp activation covers 1024 columns. PV flips orientation: probs
slices are the stationary operand so the PV output is [query-partition,
channel], which makes the softmax denominator a [128,1] reciprocal plus a
per-partition tensor_scalar multiply. Queries are grouped in odd-aligned
2-row blocks (rows 2j-1, 2j): such a block's 7-row key window spans exactly
the 8 query rows covered by the existing 512-wide score tiles, so every
(block, chunk) PV matmul is a contiguous 128-column slice of one tile.
Each block then transposes its [q, ch] attention output on the PE and runs
its own output-projection matmul, streaming y out per block.
"""
import numpy as np
import ml_dtypes
from contextlib import ExitStack

import concourse.bass as bass
import concourse.bacc as bacc
import concourse.tile as tile
import concourse.mybir as mybir
from concourse.bass_utils import run_bass_kernel_spmd
from concourse.masks import make_identity

F32 = mybir.dt.float32
BF16 = mybir.dt.bfloat16

B, H, W, D = 2, 64, 64, 512
DH, NH = 64, 8
S = H * W              # 4096 tokens per batch
KER = 7
SCALE = DH ** -0.5     # 0.125
NCORES = 8

# u1 data sits at col 96 in the PV bank so its 4-byte span stays 8B-aligned
U1 = 96

# ---------------------------------------------------------------- geometry

def _sh(r):            # clamped window start (rows); same formula for cols
    return min(max(r - KER // 2, 0), H - KER)


def _chunks_of_row(r):  # key chunks (2 rows each) seen by query row r
    s = _sh(r)
    return list(range(s // 2, (s + KER + 1) // 2))


def _build_plan():
    """TILES: scoresT [128 keys of chunk c, qw queries at q0], paired (2i,
    2i+1) into one 2-bank psum + one exp. BLOCKS: odd-aligned 2-row query
    blocks; each (block, chunk) resolves to a contiguous 128-col slice of
    one tile."""
    tiles = []
    for c in range(32):
        q0r = min(max(2 * c - 3, 0), 56)
        tiles.append(dict(c=c, q0=q0r * 64, qw=512))
    for c in (2, 3):        # query rows 0..2 miss these chunks' main windows
        tiles.append(dict(c=c, q0=0, qw=192))
    for c in (28, 29):      # query rows 61..63
        tiles.append(dict(c=c, q0=61 * 64, qw=192))

    blocks = [dict(rows=[0])]
    for j in range(1, 32):
        blocks.append(dict(rows=[2 * j - 1, 2 * j]))
    blocks.append(dict(rows=[63]))

    seen = set()
    for blk in blocks:
        rows = blk["rows"]
        blk["q0"] = rows[0] * 64
        blk["qw"] = len(rows) * 64
        chunks = sorted({c for r in rows for c in _chunks_of_row(r)})
        segs = []       # (chunk, tile_i, tile_off)
        for c in chunks:
            cand = [i for i, t in enumerate(tiles)
                    if t["c"] == c and t["q0"] <= blk["q0"]
                    and blk["q0"] + blk["qw"] <= t["q0"] + t["qw"]]
            assert cand, (blk, c)
            segs.append((c, cand[0], blk["q0"] - tiles[cand[0]]["q0"]))
        blk["segs"] = segs
        for r in rows:
            for c in _chunks_of_row(r):
                assert (r, c) not in seen
                seen.add((r, c))
    for r in range(H):
        for c in _chunks_of_row(r):
            assert (r, c) in seen, (r, c)

    # masks per tile-pair (0/1), deduped: [128 keys, 2, 512]
    starts = np.minimum(np.maximum(np.arange(H) - KER // 2, 0), H - KER)
    valid = (np.arange(H)[None, :] >= starts[:, None]) & \
            (np.arange(H)[None, :] < starts[:, None] + KER)   # [q pos, k pos]

    def tile_mask(t):
        ktok = t["c"] * 128 + np.arange(128)
        qtok = t["q0"] + np.arange(t["qw"])
        m = np.zeros((128, 512), np.float32)
        m[:, :t["qw"]] = (valid[qtok[None, :] // 64, ktok[:, None] // 64]
                          & valid[qtok[None, :] % 64, ktok[:, None] % 64])
        return m

    mask_list, mask_ids = [], {}
    pair_mask_id = []
    for pi in range(len(tiles) // 2):
        m = np.stack([tile_mask(tiles[2 * pi]), tile_mask(tiles[2 * pi + 1])],
                     axis=1)          # [128, 2, 512]
        key = m.tobytes()
        if key not in mask_ids:
            mask_ids[key] = len(mask_list)
            mask_list.append(m)
        pair_mask_id.append(mask_ids[key])
    return tiles, blocks, pair_mask_id, np.stack(mask_list)


TILES, BLOCKS, PAIR_MASK_ID, MASKS = _build_plan()
NPM = len(MASKS)

# ---------------------------------------------------------------- device

_NC_CACHE = {}
TRACE = False          # set True (e.g. from test.py) to capture an NTFF profile
LAST_RESULTS = None    # BassKernelResults of the most recent kernel() call


def _build_module():
    nc = bacc.Bacc("TRN2", target_bir_lowering=False, debug=False,
                   num_devices=NCORES)
    xT_d = nc.dram_tensor("xT", [D, S], BF16, kind="ExternalInput")
    wq_d = nc.dram_tensor("wq", [D, 128], BF16, kind="ExternalInput")
    wk_d = nc.dram_tensor("wk", [D, 128], BF16, kind="ExternalInput")
    wv_d = nc.dram_tensor("wv", [D, 128], BF16, kind="ExternalInput")
    wo_d = nc.dram_tensor("wo", [128, 512], BF16, kind="ExternalInput")
    mk_d = nc.dram_tensor("masks", [128, NPM * 1024], BF16, kind="ExternalInput")
    y_d = nc.dram_tensor("y", [S, D], BF16, kind="ExternalOutput")

    with tile.TileContext(nc) as tc, ExitStack() as ctx:
        const = ctx.enter_context(tc.tile_pool(name="const", bufs=1))
        xT_t = const.tile([128, 4, S], BF16, tag="xT")
        xr = xT_d.ap().rearrange("(c p) t -> p c t", p=128)
        for ts in range(8):     # split so projections start early
            sl = slice(ts * 512, (ts + 1) * 512)
            nc.sync.dma_start(out=xT_t[:, :, sl], in_=xr[:, :, sl])
        wq_t = const.tile([128, 4, 128], BF16, tag="wq")
        nc.sync.dma_start(out=wq_t[:], in_=wq_d.ap().rearrange("(c p) m -> p c m", p=128))
        wk_t = const.tile([128, 4, 128], BF16, tag="wk")
        nc.sync.dma_start(out=wk_t[:], in_=wk_d.ap().rearrange("(c p) m -> p c m", p=128))
        wv_t = const.tile([128, 4, 128], BF16, tag="wv")
        nc.sync.dma_start(out=wv_t[:], in_=wv_d.ap().rearrange("(c p) m -> p c m", p=128))
        wo_t = const.tile([128, 512], BF16, tag="wo")
        nc.sync.dma_start(out=wo_t[:], in_=wo_d[:, :])
        mk_t = const.tile([128, NPM * 1024], BF16, tag="mk")
        nc.sync.dma_start(out=mk_t[:], in_=mk_d[:, :])

        qT = const.tile([128, S], BF16, tag="qT")      # [2 heads x 64e, tok]
        kT = const.tile([128, S], BF16, tag="kT")
        # V: [tok_in_chunk, chunk, 130]: cols 0:64 u0-e, 64 ones, 65:129 u1-e, 129 ones
        V = const.tile([128, 32, 130], BF16, tag="V")
        nc.gpsimd.memset(V[:], 1.0)
        ident = const.tile([128, 128], BF16, tag="ident")
        make_identity(nc, ident[:])

        # ---- phase 1: projections
        # q/k: dc-outer waves of 4 so the stationary w chunk is reused
        # across 4 matmuls (LDWEIGHTS amortized)
        with tc.tile_pool(name="pps", bufs=4, space="PSUM") as pps:
            for w_t, dst in ((wq_t, qT), (wk_t, kT)):
                for wave in range(2):
                    accs = [pps.tile([128, 512], F32, tag="acc",
                                     name=f"acc_{id(w_t)}_{wave}_{i}")
                            for i in range(4)]
                    for dc in range(4):
                        for i in range(4):
                            nb = wave * 4 + i
                            nc.tensor.matmul(accs[i][:], w_t[:, dc, :],
                                             xT_t[:, dc, nb * 512:(nb + 1) * 512],
                                             start=(dc == 0), stop=(dc == 3))
                    for i in range(4):
                        nb = wave * 4 + i
                        nc.vector.tensor_copy(dst[:, nb * 512:(nb + 1) * 512],
                                              accs[i][:])
            # V in [token, channel] layout directly: xT chunk stationary
            for vb in range(8):
                acc = pps.tile([128, 4, 128], F32, tag="vacc")
                for t4 in range(4):
                    tok0 = (vb * 4 + t4) * 128
                    for dc in range(4):
                        nc.tensor.matmul(acc[:, t4, :],
                                         xT_t[:, dc, tok0:tok0 + 128],
                                         wv_t[:, dc, :],
                                         start=(dc == 0), stop=(dc == 3))
                nc.vector.tensor_copy(V[:, vb * 4:(vb + 1) * 4, 0:64],
                                      acc[:, :, 0:64])
                nc.vector.tensor_copy(V[:, vb * 4:(vb + 1) * 4, 65:129],
                                      acc[:, :, 64:128])

        # ---- phase 2: attention + per-block output projection
        with tc.tile_pool(name="scp", bufs=2, space="PSUM") as scp, \
             tc.tile_pool(name="pvp", bufs=2, space="PSUM") as pvp, \
             tc.tile_pool(name="typ", bufs=2, space="PSUM") as typ, \
             tc.tile_pool(name="prp", bufs=6) as prp, \
             tc.tile_pool(name="aop", bufs=4) as aop, \
             tc.tile_pool(name="atp", bufs=4) as atp, \
             tc.tile_pool(name="rcp", bufs=4) as rcp, \
             tc.tile_pool(name="ysp", bufs=4) as ysp:
            emitted = {}
            alt = [0]

            def ensure_pair(u, pi):
                if (u, pi) in emitted:
                    return
                ue = slice(u * 64, u * 64 + 64)
                sc = scp.tile([128, 1024], F32, tag="sc")
                for s in (0, 1):
                    t = TILES[2 * pi + s]
                    qw, c = t["qw"], t["c"]
                    nc.tensor.matmul(sc[:, s * 512:s * 512 + qw],
                                     kT[ue, c * 128:(c + 1) * 128],
                                     qT[ue, t["q0"]:t["q0"] + qw],
                                     start=True, stop=True)
                pr = prp.tile([128, 1024], BF16, tag="pr")
                nc.scalar.activation(pr[:], sc[:],
                                     mybir.ActivationFunctionType.Exp,
                                     scale=SCALE)
                mid = PAIR_MASK_ID[pi]
                nc.vector.tensor_mul(pr[:], pr[:],
                                     mk_t[:, mid * 1024:(mid + 1) * 1024])
                emitted[(u, pi)] = pr

            for blk in BLOCKS:
                qw, q0 = blk["qw"], blk["q0"]
                for u in (0, 1):
                    for c, ti, off in blk["segs"]:
                        ensure_pair(u, ti // 2)
                pv = pvp.tile([128, 512], F32, tag="pv")
                nseg = len(blk["segs"])
                # all u0 matmuls strictly before u1: the u1 group's start=True
                # clears the whole bank's has_written bits
                for u in (0, 1):
                    u0c = 0 if u == 0 else U1
                    for si, (c, ti, off) in enumerate(blk["segs"]):
                        pr = emitted[(u, ti // 2)]
                        po = (ti % 2) * 512 + off
                        nc.tensor.matmul(pv[:qw, u0c:u0c + 65],
                                         pr[:, po:po + qw],
                                         V[:, c, u * 65:u * 65 + 65],
                                         start=(si == 0), stop=(si == nseg - 1))
                rc = rcp.tile([128, 2], F32, tag="rc")
                nc.vector.reciprocal(rc[:qw, 0:1], pv[:qw, 64:65])
                nc.vector.reciprocal(rc[:qw, 1:2], pv[:qw, U1 + 64:U1 + 65])
                ao = aop.tile([128, 128], BF16, tag="ao")
                nc.vector.tensor_scalar_mul(ao[:qw, 0:64], pv[:qw, 0:64],
                                            rc[:qw, 0:1])
                nc.vector.tensor_scalar_mul(ao[:qw, 64:128], pv[:qw, U1:U1 + 64],
                                            rc[:qw, 1:2])
                # transpose into the pv bank's free tail (f32 cols 256:320
                # viewed as 128 bf16 cols): pv data is fully consumed by now,
                # and reusing the bank frees a PSUM slot for yo double-buffering
                tr = pv[:, 256:320].bitcast(BF16)
                nc.tensor.transpose(tr[:, 0:qw], ao[:qw, :], ident[0:qw, 0:qw])
                at = atp.tile([128, 128], BF16, tag="at")
                if alt[0] % 2 == 0:
                    nc.vector.tensor_copy(at[:, 0:qw], tr[:, 0:qw])
                else:
                    nc.scalar.activation(at[:, 0:qw], tr[:, 0:qw],
                                         mybir.ActivationFunctionType.Copy)
                alt[0] += 1
                yo = typ.tile([128, 512], F32, tag="yo", name=f"yo_{blk['q0']}")
                nc.tensor.matmul(yo[:qw, :], at[:, 0:qw], wo_t[:],
                                 start=True, stop=True)
                ys = ysp.tile([128, 512], BF16, tag="ys")
                if alt[0] % 2 == 0:
                    nc.vector.tensor_copy(ys[:qw, :], yo[:qw, :])
                else:
                    nc.scalar.activation(ys[:qw, :], yo[:qw, :],
                                         mybir.ActivationFunctionType.Copy)
                alt[0] += 1
                nc.sync.dma_start(out=y_d[q0:q0 + qw, :], in_=ys[:qw, :])
    nc.compile()
    return nc


def _get_module():
    if "nc" not in _NC_CACHE:
        _NC_CACHE["nc"] = _build_module()
    return _NC_CACHE["nc"]


# ---------------------------------------------------------------- host

def kernel(x, w_qkv, w_out):
    x = np.asarray(x, np.float32)
    w_qkv = np.asarray(w_qkv, np.float32)
    w_out = np.asarray(w_out, np.float32)
    nc = _get_module()

    bf = ml_dtypes.bfloat16
    # [NPM, 128, 2, 512] -> [128, NPM*1024]: key-partition major, pairs flat
    masks_bf16 = np.ascontiguousarray(
        MASKS.transpose(1, 0, 2, 3).reshape(128, NPM * 1024)).astype(bf)
    xT = [np.ascontiguousarray(x[b].reshape(S, D).T).astype(bf) for b in range(B)]
    w_outT = np.ascontiguousarray(w_out.T).astype(bf)
    w_qkv = w_qkv.astype(bf)

    in_maps = []
    for c in range(NCORES):
        b, h0 = c // 4, 2 * (c % 4)
        f = h0 * 64
        in_maps.append({
            "xT": xT[b],
            "wq": np.ascontiguousarray(w_qkv[f:f + 128].T),
            "wk": np.ascontiguousarray(w_qkv[512 + f:512 + f + 128].T),
            "wv": np.ascontiguousarray(w_qkv[1024 + f:1024 + f + 128].T),
            "wo": w_outT[f:f + 128],
            "masks": masks_bf16,
        })
    res = run_bass_kernel_spmd(nc, in_maps, list(range(NCORES)), trace=TRACE)
    global LAST_RESULTS
    LAST_RESULTS = res
    y = np.zeros((B, S, D), np.float32)
    for c in range(NCORES):
        y[c // 4] += res.results[c]["y"].astype(np.float32)
    return y.reshape(B, H, W, D)


# revision 26
# speedup vs baseline: 1.0647x; 1.0647x over previous
"""Neighbourhood attention block (7x7 clamped window) on 8 Trainium2 cores.

Sharding: (batch, head-pair) tensor parallel. Core c handles batch b = c//4
and heads (2*(c%4), 2*(c%4)+1). Each core computes q/k/v projections for its
two heads, neighbourhood attention, and a partial output projection; host
sums the 4 bf16 partials per batch in fp32.

v2 layout: all matmul operands bf16 (FWL weight loads, halved DMA).
Scores stay in scoresT [key, query] tiles, two tiles paired per 2-bank PSUM
so one Exp activation covers 1024 columns. PV flips orientation: probs
slices are the stationary operand so the PV output is [query-partition,
channel], which makes the softmax denominator a [128,1] reciprocal plus a
per-partition tensor_scalar multiply. Queries are grouped in odd-aligned
2-row blocks (rows 2j-1, 2j): such a block's 7-row key window spans exactly
the 8 query rows covered by the existing 512-wide score tiles, so every
(block, chunk) PV matmul is a contiguous 128-column slice of one tile.
Each block then transposes its [q, ch] attention output on the PE and runs
its own output-projection matmul, streaming y out per block.
"""
import numpy as np
import ml_dtypes
from contextlib import ExitStack

import concourse.bass as bass
import concourse.bacc as bacc
import concourse.tile as tile
import concourse.mybir as mybir
from concourse.bass_utils import run_bass_kernel_spmd
from concourse.masks import make_identity

F32 = mybir.dt.float32
BF16 = mybir.dt.bfloat16

B, H, W, D = 2, 64, 64, 512
DH, NH = 64, 8
S = H * W              # 4096 tokens per batch
KER = 7
SCALE = DH ** -0.5     # 0.125
NCORES = 8

# u1 data sits at col 96 in the PV bank so its 4-byte span stays 8B-aligned
U1 = 96

# ---------------------------------------------------------------- geometry

def _sh(r):            # clamped window start (rows); same formula for cols
    return min(max(r - KER // 2, 0), H - KER)


def _chunks_of_row(r):  # key chunks (2 rows each) seen by query row r
    s = _sh(r)
    return list(range(s // 2, (s + KER + 1) // 2))


def _build_plan():
    """TILES: scoresT [128 keys of chunk c, qw queries at q0], paired (2i,
    2i+1) into one 2-bank psum + one exp. BLOCKS: odd-aligned 2-row query
    blocks; each (block, chunk) resolves to a contiguous 128-col slice of
    one tile."""
    tiles = []
    for c in range(32):
        q0r = min(max(2 * c - 3, 0), 56)
        tiles.append(dict(c=c, q0=q0r * 64, qw=512))
    for c in (2, 3):        # query rows 0..2 miss these chunks' main windows
        tiles.append(dict(c=c, q0=0, qw=192))
    for c in (28, 29):      # query rows 61..63
        tiles.append(dict(c=c, q0=61 * 64, qw=192))

    blocks = [dict(rows=[0])]
    for j in range(1, 32):
        blocks.append(dict(rows=[2 * j - 1, 2 * j]))
    blocks.append(dict(rows=[63]))

    seen = set()
    for blk in blocks:
        rows = blk["rows"]
        blk["q0"] = rows[0] * 64
        blk["qw"] = len(rows) * 64
        chunks = sorted({c for r in rows for c in _chunks_of_row(r)})
        segs = []       # (chunk, tile_i, tile_off)
        for c in chunks:
            cand = [i for i, t in enumerate(tiles)
                    if t["c"] == c and t["q0"] <= blk["q0"]
                    and blk["q0"] + blk["qw"] <= t["q0"] + t["qw"]]
            assert cand, (blk, c)
            segs.append((c, cand[0], blk["q0"] - tiles[cand[0]]["q0"]))
        blk["segs"] = segs
        for r in rows:
            for c in _chunks_of_row(r):
                assert (r, c) not in seen
                seen.add((r, c))
    for r in range(H):
        for c in _chunks_of_row(r):
            assert (r, c) in seen, (r, c)

    # masks per tile-pair (0/1), deduped: [128 keys, 2, 512]
    starts = np.minimum(np.maximum(np.arange(H) - KER // 2, 0), H - KER)
    valid = (np.arange(H)[None, :] >= starts[:, None]) & \
            (np.arange(H)[None, :] < starts[:, None] + KER)   # [q pos, k pos]

    def tile_mask(t):
        ktok = t["c"] * 128 + np.arange(128)
        qtok = t["q0"] + np.arange(t["qw"])
        m = np.zeros((128, 512), np.float32)
        m[:, :t["qw"]] = (valid[qtok[None, :] // 64, ktok[:, None] // 64]
                          & valid[qtok[None, :] % 64, ktok[:, None] % 64])
        return m

    mask_list, mask_ids = [], {}
    pair_mask_id = []
    for pi in range(len(tiles) // 2):
        m = np.stack([tile_mask(tiles[2 * pi]), tile_mask(tiles[2 * pi + 1])],
                     axis=1)          # [128, 2, 512]
        key = m.tobytes()
        if key not in mask_ids:
            mask_ids[key] = len(mask_list)
            mask_list.append(m)
        pair_mask_id.append(mask_ids[key])
    return tiles, blocks, pair_mask_id, np.stack(mask_list)


TILES, BLOCKS, PAIR_MASK_ID, MASKS = _build_plan()
NPM = len(MASKS)

# ---------------------------------------------------------------- device

_NC_CACHE = {}
TRACE = False          # set True (e.g. from test.py) to capture an NTFF profile
LAST_RESULTS = None    # BassKernelResults of the most recent kernel() call


def _build_module():
    nc = bacc.Bacc("TRN2", target_bir_lowering=False, debug=False,
                   num_devices=NCORES)
    xT_d = nc.dram_tensor("xT", [D, S], BF16, kind="ExternalInput")
    wq_d = nc.dram_tensor("wq", [D, 128], BF16, kind="ExternalInput")
    wk_d = nc.dram_tensor("wk", [D, 128], BF16, kind="ExternalInput")
    wv_d = nc.dram_tensor("wv", [D, 128], BF16, kind="ExternalInput")
    wo_d = nc.dram_tensor("wo", [128, 512], BF16, kind="ExternalInput")
    mk_d = nc.dram_tensor("masks", [128, NPM * 1024], BF16, kind="ExternalInput")
    y_d = nc.dram_tensor("y", [S, D], BF16, kind="ExternalOutput")

    with tile.TileContext(nc) as tc, ExitStack() as ctx:
        const = ctx.enter_context(tc.tile_pool(name="const", bufs=1))
        xT_t = const.tile([128, 4, S], BF16, tag="xT")
        xr = xT_d.ap().rearrange("(c p) t -> p c t", p=128)
        for ts in range(8):     # split so projections start early
            sl = slice(ts * 512, (ts + 1) * 512)
            nc.sync.dma_start(out=xT_t[:, :, sl], in_=xr[:, :, sl])
        wq_t = const.tile([128, 4, 128], BF16, tag="wq")
        nc.sync.dma_start(out=wq_t[:], in_=wq_d.ap().rearrange("(c p) m -> p c m", p=128))
        wk_t = const.tile([128, 4, 128], BF16, tag="wk")
        nc.sync.dma_start(out=wk_t[:], in_=wk_d.ap().rearrange("(c p) m -> p c m", p=128))
        wv_t = const.tile([128, 4, 128], BF16, tag="wv")
        nc.sync.dma_start(out=wv_t[:], in_=wv_d.ap().rearrange("(c p) m -> p c m", p=128))
        wo_t = const.tile([128, 512], BF16, tag="wo")
        nc.sync.dma_start(out=wo_t[:], in_=wo_d[:, :])
        mk_t = const.tile([128, NPM * 1024], BF16, tag="mk")
        nc.sync.dma_start(out=mk_t[:], in_=mk_d[:, :])

        qT = const.tile([128, S], BF16, tag="qT")      # [2 heads x 64e, tok]
        kT = const.tile([128, S], BF16, tag="kT")
        # V: [tok_in_chunk, chunk, 130]: cols 0:64 u0-e, 64 ones, 65:129 u1-e, 129 ones
        V = const.tile([128, 32, 130], BF16, tag="V")
        nc.gpsimd.memset(V[:], 1.0)
        ident = const.tile([128, 128], BF16, tag="ident")
        make_identity(nc, ident[:])

        # ---- phase 1: projections
        # q/k: dc-outer waves of 4 so the stationary w chunk is reused
        # across 4 matmuls (LDWEIGHTS amortized)
        with tc.tile_pool(name="pps", bufs=4, space="PSUM") as pps:
            for w_t, dst in ((wq_t, qT), (wk_t, kT)):
                for wave in range(2):
                    accs = [pps.tile([128, 512], F32, tag="acc",
                                     name=f"acc_{id(w_t)}_{wave}_{i}")
                            for i in range(4)]
                    for dc in range(4):
                        for i in range(4):
                            nb = wave * 4 + i
                            nc.tensor.matmul(accs[i][:], w_t[:, dc, :],
                                             xT_t[:, dc, nb * 512:(nb + 1) * 512],
                                             start=(dc == 0), stop=(dc == 3))
                    for i in range(4):
                        nb = wave * 4 + i
                        nc.vector.tensor_copy(dst[:, nb * 512:(nb + 1) * 512],
                                              accs[i][:])
            # V in [token, channel] layout directly: xT chunk stationary
            for vb in range(8):
                acc = pps.tile([128, 4, 128], F32, tag="vacc")
                for t4 in range(4):
                    tok0 = (vb * 4 + t4) * 128
                    for dc in range(4):
                        nc.tensor.matmul(acc[:, t4, :],
                                         xT_t[:, dc, tok0:tok0 + 128],
                                         wv_t[:, dc, :],
                                         start=(dc == 0), stop=(dc == 3))
                nc.vector.tensor_copy(V[:, vb * 4:(vb + 1) * 4, 0:64],
                                      acc[:, :, 0:64])
                nc.vector.tensor_copy(V[:, vb * 4:(vb + 1) * 4, 65:129],
                                      acc[:, :, 64:128])

        # ---- phase 2: attention + per-block output projection
        with tc.tile_pool(name="scp", bufs=2, space="PSUM") as scp, \
             tc.tile_pool(name="pvp", bufs=2, space="PSUM") as pvp, \
             tc.tile_pool(name="typ", bufs=1, space="PSUM") as typ, \
             tc.tile_pool(name="prp", bufs=6) as prp, \
             tc.tile_pool(name="aop", bufs=4) as aop, \
             tc.tile_pool(name="atp", bufs=4) as atp, \
             tc.tile_pool(name="rcp", bufs=4) as rcp, \
             tc.tile_pool(name="ysp", bufs=4) as ysp:
            emitted = {}
            alt = [0]

            def ensure_pair(u, pi):
                if (u, pi) in emitted:
                    return
                ue = slice(u * 64, u * 64 + 64)
                sc = scp.tile([128, 1024], F32, tag="sc")
                for s in (0, 1):
                    t = TILES[2 * pi + s]
                    qw, c = t["qw"], t["c"]
                    nc.tensor.matmul(sc[:, s * 512:s * 512 + qw],
                                     kT[ue, c * 128:(c + 1) * 128],
                                     qT[ue, t["q0"]:t["q0"] + qw],
                                     start=True, stop=True)
                pr = prp.tile([128, 1024], BF16, tag="pr")
                nc.scalar.activation(pr[:], sc[:],
                                     mybir.ActivationFunctionType.Exp,
                                     scale=SCALE)
                mid = PAIR_MASK_ID[pi]
                eng = nc.gpsimd if alt[0] % 3 == 2 else nc.vector
                alt[0] += 1
                eng.tensor_mul(pr[:], pr[:],
                               mk_t[:, mid * 1024:(mid + 1) * 1024])
                emitted[(u, pi)] = pr

            for blk in BLOCKS:
                qw, q0 = blk["qw"], blk["q0"]
                for u in (0, 1):
                    for c, ti, off in blk["segs"]:
                        ensure_pair(u, ti // 2)
                pv = pvp.tile([128, 512], F32, tag="pv")
                nseg = len(blk["segs"])
                # all u0 matmuls strictly before u1: the u1 group's start=True
                # clears the whole bank's has_written bits
                for u in (0, 1):
                    u0c = 0 if u == 0 else U1
                    for si, (c, ti, off) in enumerate(blk["segs"]):
                        pr = emitted[(u, ti // 2)]
                        po = (ti % 2) * 512 + off
                        nc.tensor.matmul(pv[:qw, u0c:u0c + 65],
                                         pr[:, po:po + qw],
                                         V[:, c, u * 65:u * 65 + 65],
                                         start=(si == 0), stop=(si == nseg - 1))
                rc = rcp.tile([128, 2], F32, tag="rc")
                nc.vector.reciprocal(rc[:qw, 0:1], pv[:qw, 64:65])
                nc.vector.reciprocal(rc[:qw, 1:2], pv[:qw, U1 + 64:U1 + 65])
                ao = aop.tile([128, 128], BF16, tag="ao")
                nc.vector.tensor_scalar_mul(ao[:qw, 0:64], pv[:qw, 0:64],
                                            rc[:qw, 0:1])
                nc.vector.tensor_scalar_mul(ao[:qw, 64:128], pv[:qw, U1:U1 + 64],
                                            rc[:qw, 1:2])
                tr = typ.tile([128, 1024], BF16, tag="tr", name=f"tr_{blk['q0']}")
                nc.tensor.transpose(tr[:, 0:qw], ao[:qw, :], ident[0:qw, 0:qw])
                at = atp.tile([128, 128], BF16, tag="at")
                nc.vector.tensor_copy(at[:, 0:qw], tr[:, 0:qw])
                yo = typ.tile([128, 512], F32, tag="yo", name=f"yo_{blk['q0']}")
                nc.tensor.matmul(yo[:qw, :], at[:, 0:qw], wo_t[:],
                                 start=True, stop=True)
                ys = ysp.tile([128, 512], BF16, tag="ys")
                if alt[0] % 2 == 0:
                    nc.vector.tensor_copy(ys[:qw, :], yo[:qw, :])
                else:
                    nc.scalar.activation(ys[:qw, :], yo[:qw, :],
                                         mybir.ActivationFunctionType.Copy)
                alt[0] += 1
                nc.sync.dma_start(out=y_d[q0:q0 + qw, :], in_=ys[:qw, :])
    nc.compile()
    return nc


def _get_module():
    if "nc" not in _NC_CACHE:
        _NC_CACHE["nc"] = _build_module()
    return _NC_CACHE["nc"]


# ---------------------------------------------------------------- host

def kernel(x, w_qkv, w_out):
    x = np.asarray(x, np.float32)
    w_qkv = np.asarray(w_qkv, np.float32)
    w_out = np.asarray(w_out, np.float32)
    nc = _get_module()

    bf = ml_dtypes.bfloat16
    # [NPM, 128, 2, 512] -> [128, NPM*1024]: key-partition major, pairs flat
    masks_bf16 = np.ascontiguousarray(
        MASKS.transpose(1, 0, 2, 3).reshape(128, NPM * 1024)).astype(bf)
    xT = [np.ascontiguousarray(x[b].reshape(S, D).T).astype(bf) for b in range(B)]
    w_outT = np.ascontiguousarray(w_out.T).astype(bf)
    w_qkv = w_qkv.astype(bf)

    in_maps = []
    for c in range(NCORES):
        b, h0 = c // 4, 2 * (c % 4)
        f = h0 * 64
        in_maps.append({
            "xT": xT[b],
            "wq": np.ascontiguousarray(w_qkv[f:f + 128].T),
            "wk": np.ascontiguousarray(w_qkv[512 + f:512 + f + 128].T),
            "wv": np.ascontiguousarray(w_qkv[1024 + f:1024 + f + 128].T),
            "wo": w_outT[f:f + 128],
            "masks": masks_bf16,
        })
    res = run_bass_kernel_spmd(nc, in_maps, list(range(NCORES)), trace=TRACE)
    global LAST_RESULTS
    LAST_RESULTS = res
    y = np.zeros((B, S, D), np.float32)
    for c in range(NCORES):
        y[c // 4] += res.results[c]["y"].astype(np.float32)
    return y.reshape(B, H, W, D)


# revision 27
# speedup vs baseline: 1.1535x; 1.0834x over previous
"""Neighbourhood attention block (7x7 clamped window) on 8 Trainium2 cores.

Sharding: (batch, head-pair) tensor parallel. Core c handles batch b = c//4
and heads (2*(c%4), 2*(c%4)+1). Each core computes q/k/v projections for its
two heads, neighbourhood attention, and a partial output projection; host
sums the 4 bf16 partials per batch in fp32.

v2 layout: all matmul operands bf16 (FWL weight loads, halved DMA).
Scores stay in scoresT [key, query] tiles, two tiles paired per 2-bank PSUM
so one Exp activation covers 1024 columns. PV flips orientation: probs
slices are the stationary operand so the PV output is [query-partition,
channel], which makes the softmax denominator a [128,1] reciprocal plus a
per-partition tensor_scalar multiply. Queries are grouped in odd-aligned
2-row blocks (rows 2j-1, 2j): such a block's 7-row key window spans exactly
the 8 query rows covered by the existing 512-wide score tiles, so every
(block, chunk) PV matmul is a contiguous 128-column slice of one tile.
Each block then transposes its [q, ch] attention output on the PE and runs
its own output-projection matmul, streaming y out per block.
"""
import numpy as np
import ml_dtypes
from contextlib import ExitStack

import concourse.bass as bass
import concourse.bacc as bacc
import concourse.tile as tile
import concourse.mybir as mybir
from concourse.bass_utils import run_bass_kernel_spmd
from concourse.masks import make_identity

F32 = mybir.dt.float32
BF16 = mybir.dt.bfloat16

B, H, W, D = 2, 64, 64, 512
DH, NH = 64, 8
S = H * W              # 4096 tokens per batch
KER = 7
SCALE = DH ** -0.5     # 0.125
NCORES = 8

# u1 data sits at col 96 in the PV bank so its 4-byte span stays 8B-aligned
U1 = 96

# ---------------------------------------------------------------- geometry

def _sh(r):            # clamped window start (rows); same formula for cols
    return min(max(r - KER // 2, 0), H - KER)


def _chunks_of_row(r):  # key chunks (2 rows each) seen by query row r
    s = _sh(r)
    return list(range(s // 2, (s + KER + 1) // 2))


def _build_plan():
    """TILES: scoresT [128 keys of chunk c, qw queries at q0], paired (2i,
    2i+1) into one 2-bank psum + one exp. BLOCKS: odd-aligned 2-row query
    blocks; each (block, chunk) resolves to a contiguous 128-col slice of
    one tile."""
    tiles = []
    for c in range(32):
        q0r = min(max(2 * c - 3, 0), 56)
        tiles.append(dict(c=c, q0=q0r * 64, qw=512))
    for c in (2, 3):        # query rows 0..2 miss these chunks' main windows
        tiles.append(dict(c=c, q0=0, qw=192))
    for c in (28, 29):      # query rows 61..63
        tiles.append(dict(c=c, q0=61 * 64, qw=192))

    blocks = [dict(rows=[0])]
    for j in range(1, 32):
        blocks.append(dict(rows=[2 * j - 1, 2 * j]))
    blocks.append(dict(rows=[63]))

    seen = set()
    for blk in blocks:
        rows = blk["rows"]
        blk["q0"] = rows[0] * 64
        blk["qw"] = len(rows) * 64
        chunks = sorted({c for r in rows for c in _chunks_of_row(r)})
        segs = []       # (chunk, tile_i, tile_off)
        for c in chunks:
            cand = [i for i, t in enumerate(tiles)
                    if t["c"] == c and t["q0"] <= blk["q0"]
                    and blk["q0"] + blk["qw"] <= t["q0"] + t["qw"]]
            assert cand, (blk, c)
            segs.append((c, cand[0], blk["q0"] - tiles[cand[0]]["q0"]))
        blk["segs"] = segs
        for r in rows:
            for c in _chunks_of_row(r):
                assert (r, c) not in seen
                seen.add((r, c))
    for r in range(H):
        for c in _chunks_of_row(r):
            assert (r, c) in seen, (r, c)

    # masks per tile-pair (0/1), deduped: [128 keys, 2, 512]
    starts = np.minimum(np.maximum(np.arange(H) - KER // 2, 0), H - KER)
    valid = (np.arange(H)[None, :] >= starts[:, None]) & \
            (np.arange(H)[None, :] < starts[:, None] + KER)   # [q pos, k pos]

    def tile_mask(t):
        ktok = t["c"] * 128 + np.arange(128)
        qtok = t["q0"] + np.arange(t["qw"])
        m = np.zeros((128, 512), np.float32)
        m[:, :t["qw"]] = (valid[qtok[None, :] // 64, ktok[:, None] // 64]
                          & valid[qtok[None, :] % 64, ktok[:, None] % 64])
        return m

    mask_list, mask_ids = [], {}
    pair_mask_id = []
    for pi in range(len(tiles) // 2):
        m = np.stack([tile_mask(tiles[2 * pi]), tile_mask(tiles[2 * pi + 1])],
                     axis=1)          # [128, 2, 512]
        key = m.tobytes()
        if key not in mask_ids:
            mask_ids[key] = len(mask_list)
            mask_list.append(m)
        pair_mask_id.append(mask_ids[key])
    return tiles, blocks, pair_mask_id, np.stack(mask_list)


TILES, BLOCKS, PAIR_MASK_ID, MASKS = _build_plan()
NPM = len(MASKS)

# ---------------------------------------------------------------- device

_NC_CACHE = {}
TRACE = False          # set True (e.g. from test.py) to capture an NTFF profile
LAST_RESULTS = None    # BassKernelResults of the most recent kernel() call


def _build_module():
    nc = bacc.Bacc("TRN2", target_bir_lowering=False, debug=False,
                   num_devices=NCORES)
    xT_d = nc.dram_tensor("xT", [D, S], BF16, kind="ExternalInput")
    wq_d = nc.dram_tensor("wq", [D, 128], BF16, kind="ExternalInput")
    wk_d = nc.dram_tensor("wk", [D, 128], BF16, kind="ExternalInput")
    wv_d = nc.dram_tensor("wv", [D, 128], BF16, kind="ExternalInput")
    wo_d = nc.dram_tensor("wo", [128, 512], BF16, kind="ExternalInput")
    mk_d = nc.dram_tensor("masks", [128, NPM * 1024], BF16, kind="ExternalInput")
    y_d = nc.dram_tensor("y", [S, D], BF16, kind="ExternalOutput")

    with tile.TileContext(nc) as tc, ExitStack() as ctx:
        const = ctx.enter_context(tc.tile_pool(name="const", bufs=1))
        # weights first (tiny, unblock first matmuls), x slices round-robin
        # across the two HWDGE queues (sync + scalar), masks last
        wq_t = const.tile([128, 4, 128], BF16, tag="wq")
        nc.sync.dma_start(out=wq_t[:], in_=wq_d.ap().rearrange("(c p) m -> p c m", p=128))
        wk_t = const.tile([128, 4, 128], BF16, tag="wk")
        nc.scalar.dma_start(out=wk_t[:], in_=wk_d.ap().rearrange("(c p) m -> p c m", p=128))
        wv_t = const.tile([128, 4, 128], BF16, tag="wv")
        nc.scalar.dma_start(out=wv_t[:], in_=wv_d.ap().rearrange("(c p) m -> p c m", p=128))
        wo_t = const.tile([128, 512], BF16, tag="wo")
        nc.scalar.dma_start(out=wo_t[:], in_=wo_d[:, :])
        xT_t = const.tile([128, 4, S], BF16, tag="xT")
        xr = xT_d.ap().rearrange("(c p) t -> p c t", p=128)
        for ts in range(8):     # split so projections start early
            sl = slice(ts * 512, (ts + 1) * 512)
            eng = nc.sync if ts % 2 == 0 else nc.scalar
            eng.dma_start(out=xT_t[:, :, sl], in_=xr[:, :, sl])
        mk_t = const.tile([128, NPM * 1024], BF16, tag="mk")
        nc.scalar.dma_start(out=mk_t[:, 0:NPM * 512], in_=mk_d[:, 0:NPM * 512])
        nc.sync.dma_start(out=mk_t[:, NPM * 512:], in_=mk_d[:, NPM * 512:])

        qT = const.tile([128, S], BF16, tag="qT")      # [2 heads x 64e, tok]
        kT = const.tile([128, S], BF16, tag="kT")
        # V: [tok_in_chunk, chunk, 130]: cols 0:64 u0-e, 64 ones, 65:129 u1-e, 129 ones
        V = const.tile([128, 32, 130], BF16, tag="V")
        nc.gpsimd.memset(V[:], 1.0)
        ident = const.tile([128, 128], BF16, tag="ident")
        make_identity(nc, ident[:])

        # ---- phase 1: projections
        # q/k: dc-outer waves of 4 so the stationary w chunk is reused
        # across 4 matmuls (LDWEIGHTS amortized)
        with tc.tile_pool(name="pps", bufs=4, space="PSUM") as pps:
            for w_t, dst in ((wq_t, qT), (wk_t, kT)):
                for wave in range(2):
                    accs = [pps.tile([128, 512], F32, tag="acc",
                                     name=f"acc_{id(w_t)}_{wave}_{i}")
                            for i in range(4)]
                    for dc in range(4):
                        for i in range(4):
                            nb = wave * 4 + i
                            nc.tensor.matmul(accs[i][:], w_t[:, dc, :],
                                             xT_t[:, dc, nb * 512:(nb + 1) * 512],
                                             start=(dc == 0), stop=(dc == 3))
                    for i in range(4):
                        nb = wave * 4 + i
                        nc.vector.tensor_copy(dst[:, nb * 512:(nb + 1) * 512],
                                              accs[i][:])
            # V in [token, channel] layout directly: xT chunk stationary
            for vb in range(8):
                acc = pps.tile([128, 4, 128], F32, tag="vacc")
                for t4 in range(4):
                    tok0 = (vb * 4 + t4) * 128
                    for dc in range(4):
                        nc.tensor.matmul(acc[:, t4, :],
                                         xT_t[:, dc, tok0:tok0 + 128],
                                         wv_t[:, dc, :],
                                         start=(dc == 0), stop=(dc == 3))
                nc.vector.tensor_copy(V[:, vb * 4:(vb + 1) * 4, 0:64],
                                      acc[:, :, 0:64])
                nc.vector.tensor_copy(V[:, vb * 4:(vb + 1) * 4, 65:129],
                                      acc[:, :, 64:128])

        # ---- phase 2: attention + per-block output projection
        with tc.tile_pool(name="scp", bufs=2, space="PSUM") as scp, \
             tc.tile_pool(name="pvp", bufs=2, space="PSUM") as pvp, \
             tc.tile_pool(name="typ", bufs=1, space="PSUM") as typ, \
             tc.tile_pool(name="prp", bufs=6) as prp, \
             tc.tile_pool(name="aop", bufs=4) as aop, \
             tc.tile_pool(name="atp", bufs=4) as atp, \
             tc.tile_pool(name="rcp", bufs=4) as rcp, \
             tc.tile_pool(name="ysp", bufs=4) as ysp:
            emitted = {}
            alt = [0]

            def ensure_pair(u, pi):
                if (u, pi) in emitted:
                    return
                ue = slice(u * 64, u * 64 + 64)
                sc = scp.tile([128, 1024], F32, tag="sc")
                for s in (0, 1):
                    t = TILES[2 * pi + s]
                    qw, c = t["qw"], t["c"]
                    nc.tensor.matmul(sc[:, s * 512:s * 512 + qw],
                                     kT[ue, c * 128:(c + 1) * 128],
                                     qT[ue, t["q0"]:t["q0"] + qw],
                                     start=True, stop=True)
                pr = prp.tile([128, 1024], BF16, tag="pr")
                nc.scalar.activation(pr[:], sc[:],
                                     mybir.ActivationFunctionType.Exp,
                                     scale=SCALE)
                mid = PAIR_MASK_ID[pi]
                eng = nc.gpsimd if alt[0] % 3 == 2 else nc.vector
                alt[0] += 1
                eng.tensor_mul(pr[:], pr[:],
                               mk_t[:, mid * 1024:(mid + 1) * 1024])
                emitted[(u, pi)] = pr

            for blk in BLOCKS:
                qw, q0 = blk["qw"], blk["q0"]
                for u in (0, 1):
                    for c, ti, off in blk["segs"]:
                        ensure_pair(u, ti // 2)
                pv = pvp.tile([128, 512], F32, tag="pv")
                nseg = len(blk["segs"])
                # all u0 matmuls strictly before u1: the u1 group's start=True
                # clears the whole bank's has_written bits
                for u in (0, 1):
                    u0c = 0 if u == 0 else U1
                    for si, (c, ti, off) in enumerate(blk["segs"]):
                        pr = emitted[(u, ti // 2)]
                        po = (ti % 2) * 512 + off
                        nc.tensor.matmul(pv[:qw, u0c:u0c + 65],
                                         pr[:, po:po + qw],
                                         V[:, c, u * 65:u * 65 + 65],
                                         start=(si == 0), stop=(si == nseg - 1))
                rc = rcp.tile([128, 2], F32, tag="rc")
                nc.vector.reciprocal(rc[:qw, 0:1], pv[:qw, 64:65])
                nc.vector.reciprocal(rc[:qw, 1:2], pv[:qw, U1 + 64:U1 + 65])
                ao = aop.tile([128, 128], BF16, tag="ao")
                nc.vector.tensor_scalar_mul(ao[:qw, 0:64], pv[:qw, 0:64],
                                            rc[:qw, 0:1])
                nc.vector.tensor_scalar_mul(ao[:qw, 64:128], pv[:qw, U1:U1 + 64],
                                            rc[:qw, 1:2])
                tr = typ.tile([128, 1024], BF16, tag="tr", name=f"tr_{blk['q0']}")
                nc.tensor.transpose(tr[:, 0:qw], ao[:qw, :], ident[0:qw, 0:qw])
                at = atp.tile([128, 128], BF16, tag="at")
                nc.vector.tensor_copy(at[:, 0:qw], tr[:, 0:qw])
                yo = typ.tile([128, 512], F32, tag="yo", name=f"yo_{blk['q0']}")
                nc.tensor.matmul(yo[:qw, :], at[:, 0:qw], wo_t[:],
                                 start=True, stop=True)
                ys = ysp.tile([128, 512], BF16, tag="ys")
                if alt[0] % 2 == 0:
                    nc.vector.tensor_copy(ys[:qw, :], yo[:qw, :])
                else:
                    nc.scalar.activation(ys[:qw, :], yo[:qw, :],
                                         mybir.ActivationFunctionType.Copy)
                alt[0] += 1
                nc.sync.dma_start(out=y_d[q0:q0 + qw, :], in_=ys[:qw, :])
    nc.compile()
    return nc


def _get_module():
    if "nc" not in _NC_CACHE:
        _NC_CACHE["nc"] = _build_module()
    return _NC_CACHE["nc"]


# ---------------------------------------------------------------- host

def kernel(x, w_qkv, w_out):
    x = np.asarray(x, np.float32)
    w_qkv = np.asarray(w_qkv, np.float32)
    w_out = np.asarray(w_out, np.float32)
    nc = _get_module()

    bf = ml_dtypes.bfloat16
    # [NPM, 128, 2, 512] -> [128, NPM*1024]: key-partition major, pairs flat
    masks_bf16 = np.ascontiguousarray(
        MASKS.transpose(1, 0, 2, 3).reshape(128, NPM * 1024)).astype(bf)
    xT = [np.ascontiguousarray(x[b].reshape(S, D).T).astype(bf) for b in range(B)]
    w_outT = np.ascontiguousarray(w_out.T).astype(bf)
    w_qkv = w_qkv.astype(bf)

    in_maps = []
    for c in range(NCORES):
        b, h0 = c // 4, 2 * (c % 4)
        f = h0 * 64
        in_maps.append({
            "xT": xT[b],
            "wq": np.ascontiguousarray(w_qkv[f:f + 128].T),
            "wk": np.ascontiguousarray(w_qkv[512 + f:512 + f + 128].T),
            "wv": np.ascontiguousarray(w_qkv[1024 + f:1024 + f + 128].T),
            "wo": w_outT[f:f + 128],
            "masks": masks_bf16,
        })
    res = run_bass_kernel_spmd(nc, in_maps, list(range(NCORES)), trace=TRACE)
    global LAST_RESULTS
    LAST_RESULTS = res
    y = np.zeros((B, S, D), np.float32)
    for c in range(NCORES):
        y[c // 4] += res.results[c]["y"].astype(np.float32)
    return y.reshape(B, H, W, D)


# revision 29
# speedup vs baseline: 1.1645x; 1.0095x over previous
"""Neighbourhood attention block (7x7 clamped window) on 8 Trainium2 cores.

Sharding: (batch, head-pair) tensor parallel. Core c handles batch b = c//4
and heads (2*(c%4), 2*(c%4)+1). Each core computes q/k/v projections for its
two heads, neighbourhood attention, and a partial output projection; host
sums the 4 bf16 partials per batch in fp32.

v2 layout: all matmul operands bf16 (FWL weight loads, halved DMA).
Scores stay in scoresT [key, query] tiles, two tiles paired per 2-bank PSUM
so one Exp activation covers 1024 columns. PV flips orientation: probs
slices are the stationary operand so the PV output is [query-partition,
channel], which makes the softmax denominator a [128,1] reciprocal plus a
per-partition tensor_scalar multiply. Queries are grouped in odd-aligned
2-row blocks (rows 2j-1, 2j): such a block's 7-row key window spans exactly
the 8 query rows covered by the existing 512-wide score tiles, so every
(block, chunk) PV matmul is a contiguous 128-column slice of one tile.
Each block then transposes its [q, ch] attention output on the PE and runs
its own output-projection matmul, streaming y out per block.
"""
import numpy as np
import ml_dtypes
from contextlib import ExitStack

import concourse.bass as bass
import concourse.bacc as bacc
import concourse.tile as tile
import concourse.mybir as mybir
from concourse.bass_utils import run_bass_kernel_spmd
from concourse.masks import make_identity

F32 = mybir.dt.float32
BF16 = mybir.dt.bfloat16

B, H, W, D = 2, 64, 64, 512
DH, NH = 64, 8
S = H * W              # 4096 tokens per batch
KER = 7
SCALE = DH ** -0.5     # 0.125
NCORES = 8

# u1 data sits at col 96 in the PV bank so its 4-byte span stays 8B-aligned
U1 = 96

# ---------------------------------------------------------------- geometry

def _sh(r):            # clamped window start (rows); same formula for cols
    return min(max(r - KER // 2, 0), H - KER)


def _chunks_of_row(r):  # key chunks (2 rows each) seen by query row r
    s = _sh(r)
    return list(range(s // 2, (s + KER + 1) // 2))


def _build_plan():
    """TILES: scoresT [128 keys of chunk c, qw queries at q0], paired (2i,
    2i+1) into one 2-bank psum + one exp. BLOCKS: odd-aligned 2-row query
    blocks; each (block, chunk) resolves to a contiguous 128-col slice of
    one tile."""
    tiles = []
    for c in range(32):
        q0r = min(max(2 * c - 3, 0), 56)
        tiles.append(dict(c=c, q0=q0r * 64, qw=512))
    for c in (2, 3):        # query rows 0..2 miss these chunks' main windows
        tiles.append(dict(c=c, q0=0, qw=192))
    for c in (28, 29):      # query rows 61..63
        tiles.append(dict(c=c, q0=61 * 64, qw=192))

    blocks = [dict(rows=[0])]
    for j in range(1, 32):
        blocks.append(dict(rows=[2 * j - 1, 2 * j]))
    blocks.append(dict(rows=[63]))

    seen = set()
    for blk in blocks:
        rows = blk["rows"]
        blk["q0"] = rows[0] * 64
        blk["qw"] = len(rows) * 64
        chunks = sorted({c for r in rows for c in _chunks_of_row(r)})
        segs = []       # (chunk, tile_i, tile_off)
        for c in chunks:
            cand = [i for i, t in enumerate(tiles)
                    if t["c"] == c and t["q0"] <= blk["q0"]
                    and blk["q0"] + blk["qw"] <= t["q0"] + t["qw"]]
            assert cand, (blk, c)
            segs.append((c, cand[0], blk["q0"] - tiles[cand[0]]["q0"]))
        blk["segs"] = segs
        for r in rows:
            for c in _chunks_of_row(r):
                assert (r, c) not in seen
                seen.add((r, c))
    for r in range(H):
        for c in _chunks_of_row(r):
            assert (r, c) in seen, (r, c)

    # masks per tile-pair (0/1), deduped: [128 keys, 2, 512]
    starts = np.minimum(np.maximum(np.arange(H) - KER // 2, 0), H - KER)
    valid = (np.arange(H)[None, :] >= starts[:, None]) & \
            (np.arange(H)[None, :] < starts[:, None] + KER)   # [q pos, k pos]

    def tile_mask(t):
        ktok = t["c"] * 128 + np.arange(128)
        qtok = t["q0"] + np.arange(t["qw"])
        m = np.zeros((128, 512), np.float32)
        m[:, :t["qw"]] = (valid[qtok[None, :] // 64, ktok[:, None] // 64]
                          & valid[qtok[None, :] % 64, ktok[:, None] % 64])
        return m

    mask_list, mask_ids = [], {}
    pair_mask_id = []
    for pi in range(len(tiles) // 2):
        m = np.stack([tile_mask(tiles[2 * pi]), tile_mask(tiles[2 * pi + 1])],
                     axis=1)          # [128, 2, 512]
        key = m.tobytes()
        if key not in mask_ids:
            mask_ids[key] = len(mask_list)
            mask_list.append(m)
        pair_mask_id.append(mask_ids[key])
    return tiles, blocks, pair_mask_id, np.stack(mask_list)


TILES, BLOCKS, PAIR_MASK_ID, MASKS = _build_plan()
NPM = len(MASKS)

# ---------------------------------------------------------------- device

_NC_CACHE = {}
TRACE = False          # set True (e.g. from test.py) to capture an NTFF profile
LAST_RESULTS = None    # BassKernelResults of the most recent kernel() call


def _build_module():
    nc = bacc.Bacc("TRN2", target_bir_lowering=False, debug=False,
                   num_devices=NCORES)
    xT_d = nc.dram_tensor("xT", [D, S], BF16, kind="ExternalInput")
    wq_d = nc.dram_tensor("wq", [D, 128], BF16, kind="ExternalInput")
    wk_d = nc.dram_tensor("wk", [D, 128], BF16, kind="ExternalInput")
    wv_d = nc.dram_tensor("wv", [D, 128], BF16, kind="ExternalInput")
    wo_d = nc.dram_tensor("wo", [128, 512], BF16, kind="ExternalInput")
    mk_d = nc.dram_tensor("masks", [128, NPM * 1024], BF16, kind="ExternalInput")
    y_d = nc.dram_tensor("y", [S, D], BF16, kind="ExternalOutput")

    with tile.TileContext(nc) as tc, ExitStack() as ctx:
        const = ctx.enter_context(tc.tile_pool(name="const", bufs=1))
        # weights first (tiny, unblock first matmuls), x slices round-robin
        # across the two HWDGE queues (sync + scalar), masks last
        wq_t = const.tile([128, 4, 128], BF16, tag="wq")
        nc.sync.dma_start(out=wq_t[:], in_=wq_d.ap().rearrange("(c p) m -> p c m", p=128))
        wk_t = const.tile([128, 4, 128], BF16, tag="wk")
        nc.scalar.dma_start(out=wk_t[:], in_=wk_d.ap().rearrange("(c p) m -> p c m", p=128))
        wv_t = const.tile([128, 4, 128], BF16, tag="wv")
        nc.scalar.dma_start(out=wv_t[:], in_=wv_d.ap().rearrange("(c p) m -> p c m", p=128))
        wo_t = const.tile([128, 512], BF16, tag="wo")
        nc.scalar.dma_start(out=wo_t[:], in_=wo_d[:, :])
        xT_t = const.tile([128, 4, S], BF16, tag="xT")
        xr = xT_d.ap().rearrange("(c p) t -> p c t", p=128)
        for ts in range(8):     # split so projections start early
            sl = slice(ts * 512, (ts + 1) * 512)
            eng = nc.sync if ts % 2 == 0 else nc.scalar
            eng.dma_start(out=xT_t[:, :, sl], in_=xr[:, :, sl])
        mk_t = const.tile([128, NPM * 1024], BF16, tag="mk")
        nc.scalar.dma_start(out=mk_t[:, 0:NPM * 512], in_=mk_d[:, 0:NPM * 512])
        nc.sync.dma_start(out=mk_t[:, NPM * 512:], in_=mk_d[:, NPM * 512:])

        qT = const.tile([128, S], BF16, tag="qT")      # [2 heads x 64e, tok]
        kT = const.tile([128, S], BF16, tag="kT")
        # V: [tok_in_chunk, chunk, 130]: cols 0:64 u0-e, 64 ones, 65:129 u1-e, 129 ones
        V = const.tile([128, 32, 130], BF16, tag="V")
        nc.gpsimd.memset(V[:], 1.0)
        ident = const.tile([128, 128], BF16, tag="ident")
        make_identity(nc, ident[:])

        # ---- phase 1: projections
        # q/k: dc-outer waves of 4 so the stationary w chunk is reused
        # across 4 matmuls (LDWEIGHTS amortized)
        with tc.tile_pool(name="pps", bufs=4, space="PSUM") as pps:
            for w_t, dst in ((wq_t, qT), (wk_t, kT)):
                for wave in range(2):
                    accs = [pps.tile([128, 512], F32, tag="acc",
                                     name=f"acc_{id(w_t)}_{wave}_{i}")
                            for i in range(4)]
                    for dc in range(4):
                        for i in range(4):
                            nb = wave * 4 + i
                            nc.tensor.matmul(accs[i][:], w_t[:, dc, :],
                                             xT_t[:, dc, nb * 512:(nb + 1) * 512],
                                             start=(dc == 0), stop=(dc == 3))
                    for i in range(4):
                        nb = wave * 4 + i
                        nc.vector.tensor_copy(dst[:, nb * 512:(nb + 1) * 512],
                                              accs[i][:])
            # V in [token, channel] layout directly: xT chunk stationary
            for vb in range(8):
                acc = pps.tile([128, 4, 128], F32, tag="vacc")
                for t4 in range(4):
                    tok0 = (vb * 4 + t4) * 128
                    for dc in range(4):
                        nc.tensor.matmul(acc[:, t4, :],
                                         xT_t[:, dc, tok0:tok0 + 128],
                                         wv_t[:, dc, :],
                                         start=(dc == 0), stop=(dc == 3))
                nc.vector.tensor_copy(V[:, vb * 4:(vb + 1) * 4, 0:64],
                                      acc[:, :, 0:64])
                nc.vector.tensor_copy(V[:, vb * 4:(vb + 1) * 4, 65:129],
                                      acc[:, :, 64:128])

        # ---- phase 2: attention + per-block output projection
        with tc.tile_pool(name="scp", bufs=2, space="PSUM") as scp, \
             tc.tile_pool(name="pvp", bufs=2, space="PSUM") as pvp, \
             tc.tile_pool(name="typ", bufs=1, space="PSUM") as typ, \
             tc.tile_pool(name="prp", bufs=6) as prp, \
             tc.tile_pool(name="aop", bufs=4) as aop, \
             tc.tile_pool(name="atp", bufs=4) as atp, \
             tc.tile_pool(name="rcp", bufs=4) as rcp, \
             tc.tile_pool(name="ysp", bufs=4) as ysp:
            emitted = {}
            alt = [0]

            def ensure_pair(u, pi):
                if (u, pi) in emitted:
                    return
                ue = slice(u * 64, u * 64 + 64)
                sc = scp.tile([128, 1024], F32, tag="sc")
                for s in (0, 1):
                    t = TILES[2 * pi + s]
                    qw, c = t["qw"], t["c"]
                    nc.tensor.matmul(sc[:, s * 512:s * 512 + qw],
                                     kT[ue, c * 128:(c + 1) * 128],
                                     qT[ue, t["q0"]:t["q0"] + qw],
                                     start=True, stop=True)
                pr = prp.tile([128, 1024], BF16, tag="pr")
                nc.scalar.activation(pr[:], sc[:],
                                     mybir.ActivationFunctionType.Exp,
                                     scale=SCALE)
                mid = PAIR_MASK_ID[pi]
                eng = nc.gpsimd if alt[0] % 3 == 2 else nc.vector
                alt[0] += 1
                eng.tensor_mul(pr[:], pr[:],
                               mk_t[:, mid * 1024:(mid + 1) * 1024])
                emitted[(u, pi)] = pr

            for blk in BLOCKS:
                qw, q0 = blk["qw"], blk["q0"]
                for u in (0, 1):
                    for c, ti, off in blk["segs"]:
                        ensure_pair(u, ti // 2)
                pv = pvp.tile([128, 512], F32, tag="pv")
                nseg = len(blk["segs"])
                # all u0 matmuls strictly before u1: the u1 group's start=True
                # clears the whole bank's has_written bits
                for u in (0, 1):
                    u0c = 0 if u == 0 else U1
                    for si, (c, ti, off) in enumerate(blk["segs"]):
                        pr = emitted[(u, ti // 2)]
                        po = (ti % 2) * 512 + off
                        nc.tensor.matmul(pv[:qw, u0c:u0c + 65],
                                         pr[:, po:po + qw],
                                         V[:, c, u * 65:u * 65 + 65],
                                         start=(si == 0), stop=(si == nseg - 1))
                rc = rcp.tile([128, 2], F32, tag="rc")
                nc.vector.reciprocal(rc[:qw, 0:1], pv[:qw, 64:65])
                nc.vector.reciprocal(rc[:qw, 1:2], pv[:qw, U1 + 64:U1 + 65])
                ao = aop.tile([128, 128], BF16, tag="ao")
                nc.vector.tensor_scalar_mul(ao[:qw, 0:64], pv[:qw, 0:64],
                                            rc[:qw, 0:1])
                nc.vector.tensor_scalar_mul(ao[:qw, 64:128], pv[:qw, U1:U1 + 64],
                                            rc[:qw, 1:2])
                tr = typ.tile([128, 1024], BF16, tag="tr", name=f"tr_{blk['q0']}")
                nc.tensor.transpose(tr[:, 0:qw], ao[:qw, :], ident[0:qw, 0:qw])
                at = atp.tile([128, 128], BF16, tag="at")
                nc.vector.tensor_copy(at[:, 0:qw], tr[:, 0:qw])
                yo = typ.tile([128, 512], F32, tag="yo", name=f"yo_{blk['q0']}")
                nc.tensor.matmul(yo[:qw, :], at[:, 0:qw], wo_t[:],
                                 start=True, stop=True)
                ys = ysp.tile([128, 512], BF16, tag="ys")
                if alt[0] % 2 == 0:
                    nc.vector.tensor_copy(ys[:qw, :], yo[:qw, :])
                else:
                    nc.scalar.activation(ys[:qw, :], yo[:qw, :],
                                         mybir.ActivationFunctionType.Copy)
                yeng = nc.sync if alt[0] % 2 == 0 else nc.gpsimd
                yeng.dma_start(out=y_d[q0:q0 + qw, :], in_=ys[:qw, :])
                alt[0] += 1
    nc.compile()
    return nc


def _get_module():
    if "nc" not in _NC_CACHE:
        _NC_CACHE["nc"] = _build_module()
    return _NC_CACHE["nc"]


# ---------------------------------------------------------------- host

def kernel(x, w_qkv, w_out):
    x = np.asarray(x, np.float32)
    w_qkv = np.asarray(w_qkv, np.float32)
    w_out = np.asarray(w_out, np.float32)
    nc = _get_module()

    bf = ml_dtypes.bfloat16
    # [NPM, 128, 2, 512] -> [128, NPM*1024]: key-partition major, pairs flat
    masks_bf16 = np.ascontiguousarray(
        MASKS.transpose(1, 0, 2, 3).reshape(128, NPM * 1024)).astype(bf)
    xT = [np.ascontiguousarray(x[b].reshape(S, D).T).astype(bf) for b in range(B)]
    w_outT = np.ascontiguousarray(w_out.T).astype(bf)
    w_qkv = w_qkv.astype(bf)

    in_maps = []
    for c in range(NCORES):
        b, h0 = c // 4, 2 * (c % 4)
        f = h0 * 64
        in_maps.append({
            "xT": xT[b],
            "wq": np.ascontiguousarray(w_qkv[f:f + 128].T),
            "wk": np.ascontiguousarray(w_qkv[512 + f:512 + f + 128].T),
            "wv": np.ascontiguousarray(w_qkv[1024 + f:1024 + f + 128].T),
            "wo": w_outT[f:f + 128],
            "masks": masks_bf16,
        })
    res = run_bass_kernel_spmd(nc, in_maps, list(range(NCORES)), trace=TRACE)
    global LAST_RESULTS
    LAST_RESULTS = res
    y = np.zeros((B, S, D), np.float32)
    for c in range(NCORES):
        y[c // 4] += res.results[c]["y"].astype(np.float32)
    return y.reshape(B, H, W, D)


# revision 30
# speedup vs baseline: 1.1716x; 1.0061x over previous
"""Neighbourhood attention block (7x7 clamped window) on 8 Trainium2 cores.

Sharding: (batch, head-pair) tensor parallel. Core c handles batch b = c//4
and heads (2*(c%4), 2*(c%4)+1). Each core computes q/k/v projections for its
two heads, neighbourhood attention, and a partial output projection; host
sums the 4 bf16 partials per batch in fp32.

v2 layout: all matmul operands bf16 (FWL weight loads, halved DMA).
Scores stay in scoresT [key, query] tiles, two tiles paired per 2-bank PSUM
so one Exp activation covers 1024 columns. PV flips orientation: probs
slices are the stationary operand so the PV output is [query-partition,
channel], which makes the softmax denominator a [128,1] reciprocal plus a
per-partition tensor_scalar multiply. Queries are grouped in odd-aligned
2-row blocks (rows 2j-1, 2j): such a block's 7-row key window spans exactly
the 8 query rows covered by the existing 512-wide score tiles, so every
(block, chunk) PV matmul is a contiguous 128-column slice of one tile.
Each block then transposes its [q, ch] attention output on the PE and runs
its own output-projection matmul, streaming y out per block.
"""
import numpy as np
import ml_dtypes
from contextlib import ExitStack

import concourse.bass as bass
import concourse.bacc as bacc
import concourse.tile as tile
import concourse.mybir as mybir
from concourse.bass_utils import run_bass_kernel_spmd
from concourse.masks import make_identity

F32 = mybir.dt.float32
BF16 = mybir.dt.bfloat16

B, H, W, D = 2, 64, 64, 512
DH, NH = 64, 8
S = H * W              # 4096 tokens per batch
KER = 7
SCALE = DH ** -0.5     # 0.125
NCORES = 8

# u1 data sits at col 96 in the PV bank so its 4-byte span stays 8B-aligned
U1 = 96

# ---------------------------------------------------------------- geometry

def _sh(r):            # clamped window start (rows); same formula for cols
    return min(max(r - KER // 2, 0), H - KER)


def _chunks_of_row(r):  # key chunks (2 rows each) seen by query row r
    s = _sh(r)
    return list(range(s // 2, (s + KER + 1) // 2))


def _build_plan():
    """TILES: scoresT [128 keys of chunk c, qw queries at q0], paired (2i,
    2i+1) into one 2-bank psum + one exp. BLOCKS: odd-aligned 2-row query
    blocks; each (block, chunk) resolves to a contiguous 128-col slice of
    one tile."""
    tiles = []
    for c in range(32):
        q0r = min(max(2 * c - 3, 0), 56)
        tiles.append(dict(c=c, q0=q0r * 64, qw=512))
    for c in (2, 3):        # query rows 0..2 miss these chunks' main windows
        tiles.append(dict(c=c, q0=0, qw=192))
    for c in (28, 29):      # query rows 61..63
        tiles.append(dict(c=c, q0=61 * 64, qw=192))

    blocks = [dict(rows=[0])]
    for j in range(1, 32):
        blocks.append(dict(rows=[2 * j - 1, 2 * j]))
    blocks.append(dict(rows=[63]))

    seen = set()
    for blk in blocks:
        rows = blk["rows"]
        blk["q0"] = rows[0] * 64
        blk["qw"] = len(rows) * 64
        chunks = sorted({c for r in rows for c in _chunks_of_row(r)})
        segs = []       # (chunk, tile_i, tile_off)
        for c in chunks:
            cand = [i for i, t in enumerate(tiles)
                    if t["c"] == c and t["q0"] <= blk["q0"]
                    and blk["q0"] + blk["qw"] <= t["q0"] + t["qw"]]
            assert cand, (blk, c)
            segs.append((c, cand[0], blk["q0"] - tiles[cand[0]]["q0"]))
        blk["segs"] = segs
        for r in rows:
            for c in _chunks_of_row(r):
                assert (r, c) not in seen
                seen.add((r, c))
    for r in range(H):
        for c in _chunks_of_row(r):
            assert (r, c) in seen, (r, c)

    # masks per tile-pair (0/1), deduped: [128 keys, 2, 512]
    starts = np.minimum(np.maximum(np.arange(H) - KER // 2, 0), H - KER)
    valid = (np.arange(H)[None, :] >= starts[:, None]) & \
            (np.arange(H)[None, :] < starts[:, None] + KER)   # [q pos, k pos]

    def tile_mask(t):
        ktok = t["c"] * 128 + np.arange(128)
        qtok = t["q0"] + np.arange(t["qw"])
        m = np.zeros((128, 512), np.float32)
        m[:, :t["qw"]] = (valid[qtok[None, :] // 64, ktok[:, None] // 64]
                          & valid[qtok[None, :] % 64, ktok[:, None] % 64])
        return m

    mask_list, mask_ids = [], {}
    pair_mask_id = []
    for pi in range(len(tiles) // 2):
        m = np.stack([tile_mask(tiles[2 * pi]), tile_mask(tiles[2 * pi + 1])],
                     axis=1)          # [128, 2, 512]
        key = m.tobytes()
        if key not in mask_ids:
            mask_ids[key] = len(mask_list)
            mask_list.append(m)
        pair_mask_id.append(mask_ids[key])
    return tiles, blocks, pair_mask_id, np.stack(mask_list)


TILES, BLOCKS, PAIR_MASK_ID, MASKS = _build_plan()
NPM = len(MASKS)

# ---------------------------------------------------------------- device

_NC_CACHE = {}
TRACE = False          # set True (e.g. from test.py) to capture an NTFF profile
LAST_RESULTS = None    # BassKernelResults of the most recent kernel() call


def _build_module():
    nc = bacc.Bacc("TRN2", target_bir_lowering=False, debug=False,
                   num_devices=NCORES)
    xT_d = nc.dram_tensor("xT", [D, S], BF16, kind="ExternalInput")
    wq_d = nc.dram_tensor("wq", [D, 128], BF16, kind="ExternalInput")
    wk_d = nc.dram_tensor("wk", [D, 128], BF16, kind="ExternalInput")
    wv_d = nc.dram_tensor("wv", [D, 128], BF16, kind="ExternalInput")
    wo_d = nc.dram_tensor("wo", [128, 512], BF16, kind="ExternalInput")
    mk_d = nc.dram_tensor("masks", [128, NPM * 1024], BF16, kind="ExternalInput")
    y_d = nc.dram_tensor("y", [S, D], BF16, kind="ExternalOutput")

    with tile.TileContext(nc) as tc, ExitStack() as ctx:
        const = ctx.enter_context(tc.tile_pool(name="const", bufs=1))
        # weights first (tiny, unblock first matmuls), x slices round-robin
        # across the two HWDGE queues (sync + scalar), masks last
        wq_t = const.tile([128, 4, 128], BF16, tag="wq")
        nc.sync.dma_start(out=wq_t[:], in_=wq_d.ap().rearrange("(c p) m -> p c m", p=128))
        xT_t = const.tile([128, 4, S], BF16, tag="xT")
        xr = xT_d.ap().rearrange("(c p) t -> p c t", p=128)
        for ts in range(4):     # wave-0 x slices first, split across queues
            sl = slice(ts * 512, (ts + 1) * 512)
            eng = nc.sync if ts % 2 == 0 else nc.scalar
            eng.dma_start(out=xT_t[:, :, sl], in_=xr[:, :, sl])
        wk_t = const.tile([128, 4, 128], BF16, tag="wk")
        nc.scalar.dma_start(out=wk_t[:], in_=wk_d.ap().rearrange("(c p) m -> p c m", p=128))
        for ts in range(4, 8):
            sl = slice(ts * 512, (ts + 1) * 512)
            eng = nc.sync if ts % 2 == 0 else nc.scalar
            eng.dma_start(out=xT_t[:, :, sl], in_=xr[:, :, sl])
        wv_t = const.tile([128, 4, 128], BF16, tag="wv")
        nc.scalar.dma_start(out=wv_t[:], in_=wv_d.ap().rearrange("(c p) m -> p c m", p=128))
        wo_t = const.tile([128, 512], BF16, tag="wo")
        nc.scalar.dma_start(out=wo_t[:], in_=wo_d[:, :])
        mk_t = const.tile([128, NPM * 1024], BF16, tag="mk")
        nc.scalar.dma_start(out=mk_t[:, 0:NPM * 512], in_=mk_d[:, 0:NPM * 512])
        nc.sync.dma_start(out=mk_t[:, NPM * 512:], in_=mk_d[:, NPM * 512:])

        qT = const.tile([128, S], BF16, tag="qT")      # [2 heads x 64e, tok]
        kT = const.tile([128, S], BF16, tag="kT")
        # V: [tok_in_chunk, chunk, 130]: cols 0:64 u0-e, 64 ones, 65:129 u1-e, 129 ones
        V = const.tile([128, 32, 130], BF16, tag="V")
        nc.gpsimd.memset(V[:], 1.0)
        ident = const.tile([128, 128], BF16, tag="ident")
        make_identity(nc, ident[:])

        # ---- phase 1: projections
        # q/k: dc-outer waves of 4 so the stationary w chunk is reused
        # across 4 matmuls (LDWEIGHTS amortized)
        with tc.tile_pool(name="pps", bufs=4, space="PSUM") as pps:
            for w_t, dst in ((wq_t, qT), (wk_t, kT)):
                for wave in range(2):
                    accs = [pps.tile([128, 512], F32, tag="acc",
                                     name=f"acc_{id(w_t)}_{wave}_{i}")
                            for i in range(4)]
                    for dc in range(4):
                        for i in range(4):
                            nb = wave * 4 + i
                            nc.tensor.matmul(accs[i][:], w_t[:, dc, :],
                                             xT_t[:, dc, nb * 512:(nb + 1) * 512],
                                             start=(dc == 0), stop=(dc == 3))
                    for i in range(4):
                        nb = wave * 4 + i
                        nc.vector.tensor_copy(dst[:, nb * 512:(nb + 1) * 512],
                                              accs[i][:])
            # V in [token, channel] layout directly: xT chunk stationary
            for vb in range(8):
                acc = pps.tile([128, 4, 128], F32, tag="vacc")
                for t4 in range(4):
                    tok0 = (vb * 4 + t4) * 128
                    for dc in range(4):
                        nc.tensor.matmul(acc[:, t4, :],
                                         xT_t[:, dc, tok0:tok0 + 128],
                                         wv_t[:, dc, :],
                                         start=(dc == 0), stop=(dc == 3))
                nc.vector.tensor_copy(V[:, vb * 4:(vb + 1) * 4, 0:64],
                                      acc[:, :, 0:64])
                nc.vector.tensor_copy(V[:, vb * 4:(vb + 1) * 4, 65:129],
                                      acc[:, :, 64:128])

        # ---- phase 2: attention + per-block output projection
        with tc.tile_pool(name="scp", bufs=2, space="PSUM") as scp, \
             tc.tile_pool(name="pvp", bufs=2, space="PSUM") as pvp, \
             tc.tile_pool(name="typ", bufs=1, space="PSUM") as typ, \
             tc.tile_pool(name="prp", bufs=6) as prp, \
             tc.tile_pool(name="aop", bufs=4) as aop, \
             tc.tile_pool(name="atp", bufs=4) as atp, \
             tc.tile_pool(name="rcp", bufs=4) as rcp, \
             tc.tile_pool(name="ysp", bufs=4) as ysp:
            emitted = {}
            alt = [0]

            def ensure_pair(u, pi):
                if (u, pi) in emitted:
                    return
                ue = slice(u * 64, u * 64 + 64)
                sc = scp.tile([128, 1024], F32, tag="sc")
                for s in (0, 1):
                    t = TILES[2 * pi + s]
                    qw, c = t["qw"], t["c"]
                    nc.tensor.matmul(sc[:, s * 512:s * 512 + qw],
                                     kT[ue, c * 128:(c + 1) * 128],
                                     qT[ue, t["q0"]:t["q0"] + qw],
                                     start=True, stop=True)
                pr = prp.tile([128, 1024], BF16, tag="pr")
                nc.scalar.activation(pr[:], sc[:],
                                     mybir.ActivationFunctionType.Exp,
                                     scale=SCALE)
                mid = PAIR_MASK_ID[pi]
                eng = nc.gpsimd if alt[0] % 3 == 2 else nc.vector
                alt[0] += 1
                eng.tensor_mul(pr[:], pr[:],
                               mk_t[:, mid * 1024:(mid + 1) * 1024])
                emitted[(u, pi)] = pr

            for blk in BLOCKS:
                qw, q0 = blk["qw"], blk["q0"]
                for u in (0, 1):
                    for c, ti, off in blk["segs"]:
                        ensure_pair(u, ti // 2)
                pv = pvp.tile([128, 512], F32, tag="pv")
                nseg = len(blk["segs"])
                # all u0 matmuls strictly before u1: the u1 group's start=True
                # clears the whole bank's has_written bits
                for u in (0, 1):
                    u0c = 0 if u == 0 else U1
                    for si, (c, ti, off) in enumerate(blk["segs"]):
                        pr = emitted[(u, ti // 2)]
                        po = (ti % 2) * 512 + off
                        nc.tensor.matmul(pv[:qw, u0c:u0c + 65],
                                         pr[:, po:po + qw],
                                         V[:, c, u * 65:u * 65 + 65],
                                         start=(si == 0), stop=(si == nseg - 1))
                rc = rcp.tile([128, 2], F32, tag="rc")
                nc.vector.reciprocal(rc[:qw, 0:1], pv[:qw, 64:65])
                nc.vector.reciprocal(rc[:qw, 1:2], pv[:qw, U1 + 64:U1 + 65])
                ao = aop.tile([128, 128], BF16, tag="ao")
                nc.vector.tensor_scalar_mul(ao[:qw, 0:64], pv[:qw, 0:64],
                                            rc[:qw, 0:1])
                nc.vector.tensor_scalar_mul(ao[:qw, 64:128], pv[:qw, U1:U1 + 64],
                                            rc[:qw, 1:2])
                tr = typ.tile([128, 1024], BF16, tag="tr", name=f"tr_{blk['q0']}")
                nc.tensor.transpose(tr[:, 0:qw], ao[:qw, :], ident[0:qw, 0:qw])
                at = atp.tile([128, 128], BF16, tag="at")
                nc.vector.tensor_copy(at[:, 0:qw], tr[:, 0:qw])
                yo = typ.tile([128, 512], F32, tag="yo", name=f"yo_{blk['q0']}")
                nc.tensor.matmul(yo[:qw, :], at[:, 0:qw], wo_t[:],
                                 start=True, stop=True)
                ys = ysp.tile([128, 512], BF16, tag="ys")
                if alt[0] % 2 == 0:
                    nc.vector.tensor_copy(ys[:qw, :], yo[:qw, :])
                else:
                    nc.scalar.activation(ys[:qw, :], yo[:qw, :],
                                         mybir.ActivationFunctionType.Copy)
                yeng = nc.sync if alt[0] % 2 == 0 else nc.gpsimd
                yeng.dma_start(out=y_d[q0:q0 + qw, :], in_=ys[:qw, :])
                alt[0] += 1
    nc.compile()
    return nc


def _get_module():
    if "nc" not in _NC_CACHE:
        _NC_CACHE["nc"] = _build_module()
    return _NC_CACHE["nc"]


# ---------------------------------------------------------------- host

def kernel(x, w_qkv, w_out):
    x = np.asarray(x, np.float32)
    w_qkv = np.asarray(w_qkv, np.float32)
    w_out = np.asarray(w_out, np.float32)
    nc = _get_module()

    bf = ml_dtypes.bfloat16
    # [NPM, 128, 2, 512] -> [128, NPM*1024]: key-partition major, pairs flat
    masks_bf16 = np.ascontiguousarray(
        MASKS.transpose(1, 0, 2, 3).reshape(128, NPM * 1024)).astype(bf)
    xT = [np.ascontiguousarray(x[b].reshape(S, D).T).astype(bf) for b in range(B)]
    w_outT = np.ascontiguousarray(w_out.T).astype(bf)
    w_qkv = w_qkv.astype(bf)

    in_maps = []
    for c in range(NCORES):
        b, h0 = c // 4, 2 * (c % 4)
        f = h0 * 64
        in_maps.append({
            "xT": xT[b],
            "wq": np.ascontiguousarray(w_qkv[f:f + 128].T),
            "wk": np.ascontiguousarray(w_qkv[512 + f:512 + f + 128].T),
            "wv": np.ascontiguousarray(w_qkv[1024 + f:1024 + f + 128].T),
            "wo": w_outT[f:f + 128],
            "masks": masks_bf16,
        })
    res = run_bass_kernel_spmd(nc, in_maps, list(range(NCORES)), trace=TRACE)
    global LAST_RESULTS
    LAST_RESULTS = res
    y = np.zeros((B, S, D), np.float32)
    for c in range(NCORES):
        y[c // 4] += res.results[c]["y"].astype(np.float32)
    return y.reshape(B, H, W, D)
